# revision 27
# baseline (speedup 1.0000x reference)
"""Multi-head attention (B=128, T=256, D=512, H=8, HD=64) on 8 TRN2 NeuronCores.

Data-parallel over batch (16 batches per core), weights replicated.
Per-core Bass/Tile kernel in "transposed" space, processing batches in
PAIRS to halve instruction counts (wider matmuls, wider DVE/ACT ops):

  xT[d, (b2,t)]   <- PE-transpose of x[t, d], 2 batches packed
  QT/KT[hd,(b2,t)]<- W-pair.T @ xT   (f32r, one N=512 matmul per chunk)
  V'[s, (h,hd)]   <- xT-chunk.T @ Wv (N=512); per head the attn lhsT
                     columns are [ones | V_h] so V'.T @ expT yields the
                     softmax denominator broadcast (rows 0-63, at PSUM
                     base partition 0 where the custom-DVE approx recip
                     works) AND oT (rows 64-127) in one accumulation
                     group - no separate colsum matmul.
  scT[s, (sc,t)]  <- KT-slice.T @ QT  == scores^T       (per head/batch)
  expT            <- exp(0.125 * scT), one ACT op over both s-chunks;
                     causal mask via 2 gpsimd affine_selects (the lower
                     triangle block and the full upper s-chunk, whose
                     select also zeroes the dead quarter).
  catT            <- oT * recip(denom); recip is the single-instruction
                     Newton-Raphson approx (DVE InstReciprocal measures
                     ~5 cyc/elem on HW, ~156 us/iter; the approx is ~1)
  out[t, :]       <- catT-chunk.T @ Wo + bo (bias pre-broadcast via PE)

All f32r matmul operands are produced by compute ops (DVE/ACT copy,
activation, affine_select) to satisfy the BIR verifier rounding rule.
The next pair's load/transpose/projection work is interleaved into the
current pair's head loop to keep the PE fed.

`_emit(nc, iters=N)` repeats the whole per-dispatch pipeline N times
inside one NEFF (weights/constants loaded once).  Timing two NEFFs
(iters=n_lo vs iters=n_hi) and dividing the wall-clock difference by
(n_hi - n_lo) isolates the real steady-state hardware execution time of
one kernel iteration, cancelling the multi-millisecond axon dispatch
overhead that would otherwise dominate any wall-clock measure.
"""
from collections import deque
from contextlib import ExitStack

import numpy as np

import jax
import concourse.bass as bass
import concourse.mybir as mybir
import concourse.tile as tile
from concourse import bacc
from concourse.masks import make_identity

F32 = mybir.dt.float32
F32R = mybir.dt.float32r
EXP = mybir.ActivationFunctionType.Exp

NCORES = 8
B, T, D, H, HD = 128, 256, 512, 8, 64
BL = B // NCORES          # batches per core
NCH = D // 128            # 4 contraction chunks of 128
NPAIR = H // 2            # 4 head pairs
SCALE = float(HD) ** -0.5  # 0.125
NEXP = 5                  # expT ring depth


def _emit(nc, iters=1):
    x_d = nc.dram_tensor("x", [BL, T, D], F32, kind="ExternalInput")
    wq_d = nc.dram_tensor("Wq", [H, D, HD], F32, kind="ExternalInput")
    wk_d = nc.dram_tensor("Wk", [H, D, HD], F32, kind="ExternalInput")
    wv_d = nc.dram_tensor("Wv", [H, D, HD], F32, kind="ExternalInput")
    wo_d = nc.dram_tensor("Wo", [D, D], F32, kind="ExternalInput")
    bo_d = nc.dram_tensor("bo", [1, D], F32, kind="ExternalInput")
    out_d = nc.dram_tensor("out", [BL, T, D], F32, kind="ExternalOutput")

    NB = iters * BL           # virtual batches (x/out index = vb % BL)
    NP = NB // 2              # batch pairs

    with tile.TileContext(nc) as tc:
        with ExitStack() as ctx:
            const = ctx.enter_context(tc.tile_pool(name="const", bufs=1))
            wst = ctx.enter_context(tc.tile_pool(name="wst", bufs=2))
            xp = ctx.enter_context(tc.tile_pool(name="xp", bufs=6))
            xtp = ctx.enter_context(tc.tile_pool(name="xtp", bufs=2))
            qkvp = ctx.enter_context(tc.tile_pool(name="qkvp", bufs=2))
            recp = ctx.enter_context(tc.tile_pool(name="recp", bufs=4))
            osbp = ctx.enter_context(tc.tile_pool(name="osbp", bufs=3))
            # PSUM: 8 banks = big(2) + proj(2) + sc(2) + att(2)
            psum = ctx.enter_context(tc.tile_pool(name="ps", bufs=2, space="PSUM"))

            # ---- constants -------------------------------------------------
            ident = const.tile([128, 128], F32)
            make_identity(nc, ident)
            ones1 = const.tile([1, 128], F32)
            nc.gpsimd.memset(ones1, 1.0)
            ones_blk = const.tile([128, 2, H, HD], F32)
            nc.gpsimd.memset(ones_blk, 1.0)
            bo_sb = const.tile([1, D], F32)
            nc.sync.dma_start(bo_sb, bo_d[:, :])
            bo_ps = psum.tile([128, D], F32, tag="big", bufs=2)
            nc.tensor.matmul(bo_ps, ones1, bo_sb, start=True, stop=True)
            bo_bc = const.tile([128, D], F32)
            nc.vector.tensor_copy(bo_bc, bo_ps)

            # V ring: [s, sc, h, {V|ones}, hd]; ones half preset once
            V_bufs = []
            for i in range(4):
                vb = const.tile([128, 2, H, 2, HD], F32R, name=f"Vbuf{i}")
                nc.vector.tensor_copy(vb[:, :, :, 0, :], ones_blk)
                V_bufs.append(vb)
            # expT ring: [s, b2, 384] - 256 cols of s-chunk 0 then the
            # live (t >= 128) half of s-chunk 1
            expT_bufs = [const.tile([128, 2, 384], F32R, name=f"expT{i}")
                         for i in range(NEXP)]
            # weights loaded later (after pair-0 x DMA is queued) so the
            # first x load isn't stuck behind 16 weight-chunk DMAs
            w_r = {}
            wo_r = None

            def emit_weights():
                nonlocal wo_r
                # on the ACT hwdge queue, parallel to x loads on SP's
                for nm, wd in (("q", wq_d), ("k", wk_d), ("v", wv_d)):
                    stg = wst.tile([128, NCH, D], F32, tag="wstage",
                                   name=f"stg_{nm}")
                    wr = const.tile([128, NCH, D], F32R, name=f"w_{nm}")
                    for c in range(NCH):
                        nc.scalar.dma_start(
                            stg[:, c, :].rearrange("p (h k) -> p h k", h=H),
                            wd[:, c * 128:(c + 1) * 128, :].rearrange(
                                "h p k -> p h k"))
                        nc.vector.tensor_copy(wr[:, c, :], stg[:, c, :])
                    w_r[nm] = wr
                stg_o = wst.tile([128, NCH, D], F32, tag="wstage")
                wo_r = const.tile([128, NCH, D], F32R)
                for c in range(NCH):
                    nc.scalar.dma_start(stg_o[:, c, :],
                                        wo_d[c * 128:(c + 1) * 128, :])
                    nc.vector.tensor_copy(wo_r[:, c, :], stg_o[:, c, :])

            state = {}

            def make_pair_units(pi):
                """Closures for pair-pi prep: load, 4 transpose, 8 qk, 4 v."""
                vbs = (2 * pi) % BL, (2 * pi + 1) % BL
                units = []

                def u_load():
                    xts = []
                    for b in range(2):
                        for tci in range(2):
                            x_t = xp.tile([128, D], F32, tag="x",
                                          name=f"x_{pi}_{b}_{tci}")
                            nc.sync.dma_start(
                                x_t, x_d[vbs[b], tci * 128:(tci + 1) * 128, :])
                            xts.append(x_t)
                    xT = xtp.tile([128, NCH, 2, T], F32R, tag="xT",
                                  name=f"xT_{pi}")
                    state[pi] = {"xts": xts, "xT": xT}
                units.append(u_load)

                def u_transpose(c):
                    def f():
                        st = state[pi]
                        tp_ps = psum.tile([128, 2, 2, 128], F32, tag="big",
                                          bufs=2, name=f"tp_{pi}_{c}")
                        for b in range(2):
                            for tci in range(2):
                                nc.tensor.transpose(
                                    tp_ps[:, b, tci, :],
                                    st["xts"][2 * b + tci][
                                        :, c * 128:(c + 1) * 128],
                                    ident)
                        nc.scalar.copy(
                            st["xT"][:, c, :, :].rearrange(
                                "p b (u t) -> p b u t", u=2), tp_ps)
                    return f
                units += [u_transpose(c) for c in range(NCH)]

                def u_projqk(nm, p, dst_key):
                    def f():
                        st = state[pi]
                        if dst_key not in st:
                            st[dst_key] = qkvp.tile(
                                [128, NPAIR, 2, T], F32R, tag=dst_key,
                                name=f"{dst_key}_{pi}")
                        pj = psum.tile([128, 2, T], F32, tag="proj", bufs=2,
                                       name=f"pj_{nm}_{pi}_{p}")
                        for c in range(NCH):
                            nc.tensor.matmul(
                                pj,
                                w_r[nm][:, c, p * 128:(p + 1) * 128],
                                st["xT"][:, c, :, :],
                                start=(c == 0), stop=(c == NCH - 1))
                        nc.vector.tensor_copy(st[dst_key][:, p, :, :], pj)
                    return f
                units += [u_projqk("q", p, "QT") for p in range(NPAIR)]
                units += [u_projqk("k", p, "KT") for p in range(NPAIR)]

                def u_projv(b, sc):
                    def f():
                        st = state[pi]
                        vb = V_bufs[(2 * pi + b) % 4]
                        pj = psum.tile([128, H, HD], F32, tag="proj", bufs=2,
                                       name=f"pj_v_{pi}_{b}_{sc}")
                        for c in range(NCH):
                            nc.tensor.matmul(
                                pj,
                                st["xT"][:, c, b, sc * 128:(sc + 1) * 128],
                                w_r["v"][:, c, :],
                                start=(c == 0), stop=(c == NCH - 1))
                        nc.scalar.copy(vb[:, sc, :, 1, :], pj)
                    return f
                units += [u_projv(b, sc) for b in range(2) for sc in range(2)]
                return units

            def emit_scores(pi, h):
                st = state[pi]
                p, hh = divmod(h, 2)
                pb = hh * HD
                eb = expT_bufs[(pi * H + h) % NEXP]
                for b in range(2):
                    sc_ps = psum.tile([128, 384], F32, tag="sc", bufs=2,
                                      name=f"sc_{pi}_{h}_{b}")
                    # s-chunk 0 sees all t; s-chunk 1 only attends t >= 128,
                    # so its live half is packed right after chunk 0's 256
                    # columns - no causally-dead quarter is ever computed
                    nc.tensor.matmul(
                        sc_ps[:, 0:256],
                        st["KT"][pb:pb + HD, p, b, 0:128],
                        st["QT"][pb:pb + HD, p, b, :],
                        start=True, stop=True)
                    nc.tensor.matmul(
                        sc_ps[:, 256:384],
                        st["KT"][pb:pb + HD, p, b, 128:256],
                        st["QT"][pb:pb + HD, p, b, 128:256],
                        start=True, stop=True)
                    nc.scalar.activation(eb[:, b, :], sc_ps, EXP,
                                         scale=SCALE)
                    # causal: both s-chunks' diagonal 128x128 blocks keep
                    # t - s >= 0 in chunk-local coordinates
                    nc.gpsimd.affine_select(
                        out=eb[:, b, 0:128], in_=eb[:, b, 0:128],
                        compare_op=mybir.AluOpType.is_ge, fill=0.0,
                        base=0, pattern=[[1, 128]], channel_multiplier=-1)
                    nc.gpsimd.affine_select(
                        out=eb[:, b, 256:384], in_=eb[:, b, 256:384],
                        compare_op=mybir.AluOpType.is_ge, fill=0.0,
                        base=0, pattern=[[1, 128]], channel_multiplier=-1)
                return eb

            def emit_tail(pi, h, eb, catT):
                p, hh = divmod(h, 2)
                ot_ps = psum.tile([128, 2, T], F32, tag="att", bufs=2,
                                  name=f"ot_{pi}_{h}")
                for b in range(2):
                    vb = V_bufs[(2 * pi + b) % 4]
                    # chunk 1 contributes only to t >= 128: its matmul
                    # accumulates into the right half of chunk 0's output
                    nc.tensor.matmul(ot_ps[:, b, :],
                                     vb[:, 0, h, :, :], eb[:, b, 0:256],
                                     start=True, stop=False)
                    nc.tensor.matmul(ot_ps[:, b, 128:256],
                                     vb[:, 1, h, :, :], eb[:, b, 256:384],
                                     start=False, stop=True)
                # softmax normalize: catT = oT * approx-recip(denom).  DVE
                # InstReciprocal is ~5 cyc/elem on HW (~156 us/iter
                # measured) and the TensorTensor divide ALU op fails the
                # ISA check; the single-instruction Newton-Raphson approx
                # reciprocal (~18 bits; denom is a sum of exps in
                # [1, ~5e3], far from its 0/denorm/inf edge cases) reads
                # the denom straight from PSUM partitions 0-63 (the attn
                # lhsT is [ones | V] so the denom lands at base 0).
                recip = recp.tile([HD, 2, T], F32, tag="rec2",
                                  name=f"rec_{pi}_{h}")
                nc.vector.reciprocal_approx_fast(
                    out=recip, in_=ot_ps[0:HD, :, :])
                nc.vector.tensor_mul(catT[hh * HD:(hh + 1) * HD, p, :, :],
                                     ot_ps[HD:2 * HD, :, :], recip)

            def mk_outproj(pi, catT):
                vbs = (2 * pi) % BL, (2 * pi + 1) % BL

                def one(b, tci):
                    def f():
                        po = psum.tile([128, D], F32, tag="big", bufs=2,
                                       name=f"po_{pi}_{b}_{tci}")
                        for c in range(NCH):
                            nc.tensor.matmul(
                                po,
                                catT[:, c, b, tci * 128:(tci + 1) * 128],
                                wo_r[:, c, :],
                                start=(c == 0), stop=(c == NCH - 1))
                        osb = osbp.tile([128, D], F32, tag="osb",
                                        name=f"osb_{pi}_{b}_{tci}")
                        nc.vector.tensor_add(osb, po, bo_bc)
                        nc.sync.dma_start(
                            out_d[vbs[b], tci * 128:(tci + 1) * 128, :], osb)
                    return f
                return [one(b, tci) for b in range(2) for tci in range(2)]

            # ---- main pipeline --------------------------------------------
            fillers = deque()
            units0 = make_pair_units(0)
            for u in units0[:5]:
                u()                  # x(pair0) DMA + transposes first
            units1 = make_pair_units(1) if NP > 1 else None
            if units1:
                units1[0]()          # x(pair1) DMA also ahead of the weights
            emit_weights()           # weight DMAs on the other queue
            for u in units0[5:]:
                u()                  # pair-0 projections
            pending_out = deque()
            for pi in range(NP):
                if pi + 1 < NP:
                    fillers.extend(units1[1:] if pi == 0
                                   else make_pair_units(pi + 1))
                catT = qkvp.tile([128, NPAIR, 2, T], F32R, tag="cat",
                                 name=f"catT_{pi}")
                pend = deque()
                for i in range(H + 2):
                    if i < H:
                        pend.append((i, emit_scores(pi, i)))
                    if i >= 2:
                        hh_, eb_ = pend.popleft()
                        emit_tail(pi, hh_, eb_, catT)
                    if pending_out:
                        pending_out.popleft()()  # prev pair's out-proj
                    for _ in range(2):
                        if fillers:
                            fillers.popleft()()
                while fillers:
                    fillers.popleft()()
                pending_out.extend(mk_outproj(pi, catT))
                state.pop(pi - 1, None)
            while pending_out:
                pending_out.popleft()()

    nc.compile()
    return nc


_CACHE = {}
BENCH_LO, BENCH_HI = 3, 11


def _make_exec(iters):
    """Build the bass module for `iters` pipeline repetitions and return a
    jitted SPMD dispatcher plus metadata."""
    from jax.sharding import Mesh, PartitionSpec
    from jax.experimental.shard_map import shard_map
    from concourse.bass2jax import (
        _bass_exec_p, install_neuronx_cc_hook, partition_id_tensor)
    import concourse.mybir as mybir_

    nc = bacc.Bacc("TRN2", target_bir_lowering=False, debug=False)
    _emit(nc, iters=iters)

    install_neuronx_cc_hook()

    partition_name = (nc.partition_id_tensor.name
                      if nc.partition_id_tensor else None)
    in_names, out_names, out_avals, zero_outs = [], [], [], []
    for alloc in nc.m.functions[0].allocations:
        if not isinstance(alloc, mybir_.MemoryLocationSet):
            continue
        name = alloc.memorylocations[0].name
        if alloc.kind == "ExternalInput":
            if name != partition_name:
                in_names.append(name)
        elif alloc.kind == "ExternalOutput":
            out_names.append(name)
            shape = tuple(alloc.tensor_shape)
            dtype = mybir_.dt.np(alloc.dtype)
            out_avals.append(jax.core.ShapedArray(shape, dtype))
            zero_outs.append(np.zeros((NCORES * shape[0], *shape[1:]), dtype))
    n_params = len(in_names)
    all_names = in_names + out_names
    if partition_name is not None:
        all_names = all_names + [partition_name]

    def _body(*args):
        operands = list(args)
        if partition_name is not None:
            operands.append(partition_id_tensor())
        outs = _bass_exec_p.bind(
            *operands,
            out_avals=tuple(out_avals),
            in_names=tuple(all_names),
            out_names=tuple(out_names),
            lowering_input_output_aliases=(),
            sim_require_finite=True,
            sim_require_nnan=True,
            nc=nc,
        )
        return tuple(outs)

    devices = jax.devices()[:NCORES]
    mesh = Mesh(np.asarray(devices), ("core",))
    n_outs = len(out_names)
    # x is batch-sharded; weights are replicated (sent once, not 8x)
    spec_of = {n: (PartitionSpec("core") if n == "x" else PartitionSpec())
               for n in in_names}
    nodonate = jax.jit(
        shard_map(
            _body, mesh=mesh,
            in_specs=tuple(spec_of[n] for n in in_names)
            + (PartitionSpec("core"),) * n_outs,
            out_specs=(PartitionSpec("core"),) * n_outs,
            check_rep=False,
        ),
        keep_unused=True,
    )
    return {
        "nc": nc, "exec": nodonate, "in_names": in_names,
        "out_names": out_names, "zero_outs": zero_outs,
        "mesh": mesh, "spec_of": spec_of,
    }


def _get_exec(iters):
    key = ("exec", iters)
    if key not in _CACHE:
        _CACHE[key] = _make_exec(iters)
    return _CACHE[key]


def _device_args(ex, in_map_global):
    from jax.sharding import NamedSharding, PartitionSpec
    args = [jax.device_put(in_map_global[n],
                           NamedSharding(ex["mesh"], ex["spec_of"][n]))
            for n in ex["in_names"]]
    zs = [jax.device_put(z, NamedSharding(ex["mesh"], PartitionSpec("core")))
          for z in ex["zero_outs"]]
    return args, zs


def _time_exec(ex, args, zs, iters=20, reps=8):
    """Min pipelined per-dispatch wall time with device-resident buffers."""
    import time as _t
    f = ex["exec"]
    for _ in range(3):
        o = f(*args, *zs)
        jax.block_until_ready(o)
    runs = []
    for _ in range(reps):
        t0 = _t.perf_counter()
        for _ in range(iters):
            o = f(*args, *zs)
        jax.block_until_ready(o)
        runs.append((_t.perf_counter() - t0) / iters)
    return min(runs), runs


def bench_hw(in_map_global, sizes=(BENCH_LO, BENCH_HI), iters=20,
             rounds=8):
    """True per-iteration HW kernel time: time NEFFs that run the whole
    pipeline n times back-to-back on-device for several n and take the
    least-squares slope of wall time vs n.  The slope cancels the axon
    dispatch overhead (~4.4 ms per dispatch, kernel-independent) and
    once-per-NEFF costs (weight loads, final drain/barrier).  The NEFFs are
    timed in interleaved rounds so slow drift in the RPC floor cancels out
    of the slope; per-size minima over all rounds are used.  The 1x NEFF is
    deliberately excluded: its device time partially hides under the
    dispatch pipeline, which would bias the slope optimistically."""
    import time as _t
    exs = [(n, _get_exec(n)) for n in sizes]
    prepped = [(n, ex, *_device_args(ex, in_map_global)) for n, ex in exs]

    def once(ex, args, zs):
        f = ex["exec"]
        t0 = _t.perf_counter()
        for _ in range(iters):
            o = f(*args, *zs)
        jax.block_until_ready(o)
        return (_t.perf_counter() - t0) / iters

    for n, ex, args, zs in prepped:
        for _ in range(3):
            jax.block_until_ready(ex["exec"](*args, *zs))
    runs = {n: [] for n in sizes}
    for _ in range(rounds):
        for n, ex, args, zs in prepped:
            runs[n].append(once(ex, args, zs))
    mins = {n: min(rs) for n, rs in runs.items()}
    ns = np.array(sizes, dtype=np.float64)
    ts = np.array([mins[n] for n in sizes])
    per = float(np.polyfit(ns, ts, 1)[0])
    return per, {"mins": mins, "runs": runs, "sizes": sizes}


def _run(in_map_global):
    ex = _get_exec(1)
    args, zs = _device_args(ex, in_map_global)
    outs = ex["exec"](*args, *zs)
    return {n: np.asarray(outs[i]) for i, n in enumerate(ex["out_names"])}


def kernel(x, Wq, Wk, Wv, Wo, bo):
    in_map = {
        "x": np.ascontiguousarray(np.asarray(x, np.float32)),      # [128,256,512]
        "Wq": np.asarray(Wq, np.float32),
        "Wk": np.asarray(Wk, np.float32),
        "Wv": np.asarray(Wv, np.float32),
        "Wo": np.asarray(Wo, np.float32),
        "bo": np.asarray(bo, np.float32).reshape(1, D),
    }
    out = _run(in_map)["out"]                                      # [128,256,512]
    return out.astype(np.float32)


# revision 30
# speedup vs baseline: 1.2056x; 1.2056x over previous
"""Multi-head attention (B=128, T=256, D=512, H=8, HD=64) on 8 TRN2 NeuronCores.

Data-parallel over batch (16 batches per core), weights replicated.
Per-core Bass/Tile kernel in "transposed" space, processing batches in
PAIRS to halve instruction counts (wider matmuls, wider DVE/ACT ops):

  xT[d, (b2,t)]   <- PE-transpose of x[t, d], 2 batches packed
  QT/KT[hd,(b2,t)]<- W-pair.T @ xT   (f32r, one N=512 matmul per chunk)
  V'[s, (h,hd)]   <- xT-chunk.T @ Wv (N=512); per head the attn lhsT
                     columns are [ones | V_h] so V'.T @ expT yields the
                     softmax denominator broadcast (rows 0-63, at PSUM
                     base partition 0 where the custom-DVE approx recip
                     works) AND oT (rows 64-127) in one accumulation
                     group - no separate colsum matmul.
  scT[s, (sc,t)]  <- KT-slice.T @ QT  == scores^T       (per head/batch)
  expT            <- exp(0.125 * scT), one ACT op over both s-chunks;
                     causal mask via 2 gpsimd affine_selects (the lower
                     triangle block and the full upper s-chunk, whose
                     select also zeroes the dead quarter).
  catT            <- oT * recip(denom); recip is the single-instruction
                     Newton-Raphson approx (DVE InstReciprocal measures
                     ~5 cyc/elem on HW, ~156 us/iter; the approx is ~1)
  out[t, :]       <- catT-chunk.T @ Wo + bo (bias pre-broadcast via PE)

All f32r matmul operands are produced by compute ops (DVE/ACT copy,
activation, affine_select) to satisfy the BIR verifier rounding rule.
The next pair's load/transpose/projection work is interleaved into the
current pair's head loop to keep the PE fed.

`_emit(nc, iters=N)` repeats the whole per-dispatch pipeline N times
inside one NEFF (weights/constants loaded once).  Timing two NEFFs
(iters=n_lo vs iters=n_hi) and dividing the wall-clock difference by
(n_hi - n_lo) isolates the real steady-state hardware execution time of
one kernel iteration, cancelling the multi-millisecond axon dispatch
overhead that would otherwise dominate any wall-clock measure.
"""
from collections import deque
from contextlib import ExitStack

import numpy as np

import jax
import concourse.bass as bass
import concourse.mybir as mybir
import concourse.tile as tile
from concourse import bacc
from concourse.masks import make_identity

F32 = mybir.dt.float32
F32R = mybir.dt.float32r
EXP = mybir.ActivationFunctionType.Exp

NCORES = 8
B, T, D, H, HD = 128, 256, 512, 8, 64
BL = B // NCORES          # batches per core
NCH = D // 128            # 4 contraction chunks of 128
NPAIR = H // 2            # 4 head pairs
SCALE = float(HD) ** -0.5  # 0.125
NEXP = 5                  # expT ring depth


def _emit(nc, iters=1):
    x_d = nc.dram_tensor("x", [BL, T, D], F32, kind="ExternalInput")
    wq_d = nc.dram_tensor("Wq", [H, D, HD], F32, kind="ExternalInput")
    wk_d = nc.dram_tensor("Wk", [H, D, HD], F32, kind="ExternalInput")
    wv_d = nc.dram_tensor("Wv", [H, D, HD], F32, kind="ExternalInput")
    wo_d = nc.dram_tensor("Wo", [D, D], F32, kind="ExternalInput")
    bo_d = nc.dram_tensor("bo", [1, D], F32, kind="ExternalInput")
    out_d = nc.dram_tensor("out", [BL, T, D], F32, kind="ExternalOutput")

    NB = iters * BL           # virtual batches (x/out index = vb % BL)
    NP = NB // 2              # batch pairs

    with tile.TileContext(nc) as tc:
        with ExitStack() as ctx:
            const = ctx.enter_context(tc.tile_pool(name="const", bufs=1))
            wst = ctx.enter_context(tc.tile_pool(name="wst", bufs=2))
            xp = ctx.enter_context(tc.tile_pool(name="xp", bufs=6))
            xtp = ctx.enter_context(tc.tile_pool(name="xtp", bufs=2))
            qkvp = ctx.enter_context(tc.tile_pool(name="qkvp", bufs=2))
            recp = ctx.enter_context(tc.tile_pool(name="recp", bufs=4))
            osbp = ctx.enter_context(tc.tile_pool(name="osbp", bufs=3))
            # PSUM: 8 banks = big(2) + proj(2) + sc(2) + att(2)
            psum = ctx.enter_context(tc.tile_pool(name="ps", bufs=2, space="PSUM"))

            # ---- constants -------------------------------------------------
            ident = const.tile([128, 128], F32)
            make_identity(nc, ident)
            ones1 = const.tile([1, 128], F32)
            nc.gpsimd.memset(ones1, 1.0)
            ones_blk = const.tile([128, 2, H, HD], F32)
            nc.gpsimd.memset(ones_blk, 1.0)
            bo_sb = const.tile([1, D], F32)
            nc.sync.dma_start(bo_sb, bo_d[:, :])
            bo_ps = psum.tile([128, D], F32, tag="big", bufs=2)
            nc.tensor.matmul(bo_ps, ones1, bo_sb, start=True, stop=True)
            bo_bc = const.tile([128, D], F32)
            nc.vector.tensor_copy(bo_bc, bo_ps)

            # V ring: [s, sc, h, {V|ones}, hd]; ones half preset once
            V_bufs = []
            for i in range(4):
                vb = const.tile([128, 2, H, 2, HD], F32R, name=f"Vbuf{i}")
                nc.vector.tensor_copy(vb[:, :, :, 0, :], ones_blk)
                V_bufs.append(vb)
            # expT ring: [s, b2, 384] - 256 cols of s-chunk 0 then the
            # live (t >= 128) half of s-chunk 1
            expT_bufs = [const.tile([128, 2, 384], F32R, name=f"expT{i}")
                         for i in range(NEXP)]
            # weights loaded later (after pair-0 x DMA is queued) so the
            # first x load isn't stuck behind 16 weight-chunk DMAs
            w_r = {}
            wo_r = None

            def emit_weights():
                nonlocal wo_r
                # on the ACT hwdge queue, parallel to x loads on SP's
                for nm, wd in (("q", wq_d), ("k", wk_d), ("v", wv_d)):
                    stg = wst.tile([128, NCH, D], F32, tag="wstage",
                                   name=f"stg_{nm}")
                    wr = const.tile([128, NCH, D], F32R, name=f"w_{nm}")
                    for c in range(NCH):
                        nc.scalar.dma_start(
                            stg[:, c, :].rearrange("p (h k) -> p h k", h=H),
                            wd[:, c * 128:(c + 1) * 128, :].rearrange(
                                "h p k -> p h k"))
                        nc.vector.tensor_copy(wr[:, c, :], stg[:, c, :])
                    w_r[nm] = wr
                stg_o = wst.tile([128, NCH, D], F32, tag="wstage")
                wo_r = const.tile([128, NCH, D], F32R)
                for c in range(NCH):
                    nc.scalar.dma_start(stg_o[:, c, :],
                                        wo_d[c * 128:(c + 1) * 128, :])
                    nc.vector.tensor_copy(wo_r[:, c, :], stg_o[:, c, :])

            state = {}

            def make_pair_units(pi):
                """Closures for pair-pi prep: load, 4 transpose, 8 qk, 4 v."""
                vbs = (2 * pi) % BL, (2 * pi + 1) % BL
                units = []

                def u_load():
                    xts = []
                    for b in range(2):
                        for tci in range(2):
                            x_t = xp.tile([128, D], F32, tag="x",
                                          name=f"x_{pi}_{b}_{tci}")
                            nc.sync.dma_start(
                                x_t, x_d[vbs[b], tci * 128:(tci + 1) * 128, :])
                            xts.append(x_t)
                    xT = xtp.tile([128, NCH, 2, T], F32R, tag="xT",
                                  name=f"xT_{pi}")
                    state[pi] = {"xts": xts, "xT": xT}
                units.append(u_load)

                def u_transpose(c):
                    def f():
                        st = state[pi]
                        tp_ps = psum.tile([128, 2, 2, 128], F32, tag="big",
                                          bufs=2, name=f"tp_{pi}_{c}")
                        for b in range(2):
                            for tci in range(2):
                                nc.tensor.transpose(
                                    tp_ps[:, b, tci, :],
                                    st["xts"][2 * b + tci][
                                        :, c * 128:(c + 1) * 128],
                                    ident)
                        nc.scalar.copy(
                            st["xT"][:, c, :, :].rearrange(
                                "p b (u t) -> p b u t", u=2), tp_ps)
                    return f
                units += [u_transpose(c) for c in range(NCH)]

                def u_projqk(nm, p, dst_key):
                    def f():
                        st = state[pi]
                        if dst_key not in st:
                            st[dst_key] = qkvp.tile(
                                [128, NPAIR, 2, T], F32R, tag=dst_key,
                                name=f"{dst_key}_{pi}")
                        pj = psum.tile([128, 2, T], F32, tag="proj", bufs=2,
                                       name=f"pj_{nm}_{pi}_{p}")
                        for c in range(NCH):
                            nc.tensor.matmul(
                                pj,
                                w_r[nm][:, c, p * 128:(p + 1) * 128],
                                st["xT"][:, c, :, :],
                                start=(c == 0), stop=(c == NCH - 1))
                        nc.vector.tensor_copy(st[dst_key][:, p, :, :], pj)
                    return f
                units += [u_projqk("q", p, "QT") for p in range(NPAIR)]
                units += [u_projqk("k", p, "KT") for p in range(NPAIR)]

                def u_projv(b, sc):
                    def f():
                        st = state[pi]
                        vb = V_bufs[(2 * pi + b) % 4]
                        pj = psum.tile([128, H, HD], F32, tag="proj", bufs=2,
                                       name=f"pj_v_{pi}_{b}_{sc}")
                        for c in range(NCH):
                            nc.tensor.matmul(
                                pj,
                                st["xT"][:, c, b, sc * 128:(sc + 1) * 128],
                                w_r["v"][:, c, :],
                                start=(c == 0), stop=(c == NCH - 1))
                        nc.scalar.copy(vb[:, sc, :, 1, :], pj)
                    return f
                units += [u_projv(b, sc) for b in range(2) for sc in range(2)]
                return units

            def emit_scores(pi, h):
                st = state[pi]
                p, hh = divmod(h, 2)
                pb = hh * HD
                eb = expT_bufs[(pi * H + h) % NEXP]
                for b in range(2):
                    sc_ps = psum.tile([128, 384], F32, tag="sc", bufs=2,
                                      name=f"sc_{pi}_{h}_{b}")
                    # s-chunk 0 sees all t; s-chunk 1 only attends t >= 128,
                    # so its live half is packed right after chunk 0's 256
                    # columns - no causally-dead quarter is ever computed
                    nc.tensor.matmul(
                        sc_ps[:, 0:256],
                        st["KT"][pb:pb + HD, p, b, 0:128],
                        st["QT"][pb:pb + HD, p, b, :],
                        start=True, stop=True)
                    nc.tensor.matmul(
                        sc_ps[:, 256:384],
                        st["KT"][pb:pb + HD, p, b, 128:256],
                        st["QT"][pb:pb + HD, p, b, 128:256],
                        start=True, stop=True)
                    nc.scalar.activation(eb[:, b, :], sc_ps, EXP,
                                         scale=SCALE)
                    # causal: both s-chunks' diagonal 128x128 blocks (at
                    # offsets 0 and 256, same t - s >= 0 condition in
                    # chunk-local coordinates) in ONE strided select
                    tri = eb[:, b, :].rearrange(
                        "p (u q) -> p u q", q=128)[:, 0::2, :]
                    nc.gpsimd.affine_select(
                        out=tri, in_=tri,
                        compare_op=mybir.AluOpType.is_ge, fill=0.0,
                        base=0, pattern=[[0, 2], [1, 128]],
                        channel_multiplier=-1)
                return eb

            def emit_tail(pi, h, eb, catT):
                p, hh = divmod(h, 2)
                ot_ps = psum.tile([128, 2, T], F32, tag="att", bufs=2,
                                  name=f"ot_{pi}_{h}")
                for b in range(2):
                    vb = V_bufs[(2 * pi + b) % 4]
                    # chunk 1 contributes only to t >= 128: its matmul
                    # accumulates into the right half of chunk 0's output
                    nc.tensor.matmul(ot_ps[:, b, :],
                                     vb[:, 0, h, :, :], eb[:, b, 0:256],
                                     start=True, stop=False)
                    nc.tensor.matmul(ot_ps[:, b, 128:256],
                                     vb[:, 1, h, :, :], eb[:, b, 256:384],
                                     start=False, stop=True)
                # softmax normalize: catT = oT * approx-recip(denom).  DVE
                # InstReciprocal is ~5 cyc/elem on HW (~156 us/iter
                # measured) and the TensorTensor divide ALU op fails the
                # ISA check; the single-instruction Newton-Raphson approx
                # reciprocal (~18 bits; denom is a sum of exps in
                # [1, ~5e3], far from its 0/denorm/inf edge cases) reads
                # the denom straight from PSUM partitions 0-63 (the attn
                # lhsT is [ones | V] so the denom lands at base 0).
                recip = recp.tile([HD, 2, T], F32, tag="rec2",
                                  name=f"rec_{pi}_{h}")
                nc.vector.reciprocal_approx_fast(
                    out=recip, in_=ot_ps[0:HD, :, :])
                nc.vector.tensor_mul(catT[hh * HD:(hh + 1) * HD, p, :, :],
                                     ot_ps[HD:2 * HD, :, :], recip)

            def mk_outproj(pi, catT):
                vbs = (2 * pi) % BL, (2 * pi + 1) % BL

                def one(b, tci):
                    def f():
                        po = psum.tile([128, D], F32, tag="big", bufs=2,
                                       name=f"po_{pi}_{b}_{tci}")
                        for c in range(NCH):
                            nc.tensor.matmul(
                                po,
                                catT[:, c, b, tci * 128:(tci + 1) * 128],
                                wo_r[:, c, :],
                                start=(c == 0), stop=(c == NCH - 1))
                        osb = osbp.tile([128, D], F32, tag="osb",
                                        name=f"osb_{pi}_{b}_{tci}")
                        nc.vector.tensor_add(osb, po, bo_bc)
                        nc.sync.dma_start(
                            out_d[vbs[b], tci * 128:(tci + 1) * 128, :], osb)
                    return f
                return [one(b, tci) for b in range(2) for tci in range(2)]

            # ---- main pipeline --------------------------------------------
            fillers = deque()
            units0 = make_pair_units(0)
            for u in units0[:5]:
                u()                  # x(pair0) DMA + transposes first
            units1 = make_pair_units(1) if NP > 1 else None
            if units1:
                units1[0]()          # x(pair1) DMA also ahead of the weights
            emit_weights()           # weight DMAs on the other queue
            for u in units0[5:]:
                u()                  # pair-0 projections
            pending_out = deque()
            for pi in range(NP):
                if pi + 1 < NP:
                    fillers.extend(units1[1:] if pi == 0
                                   else make_pair_units(pi + 1))
                catT = qkvp.tile([128, NPAIR, 2, T], F32R, tag="cat",
                                 name=f"catT_{pi}")
                pend = deque()
                for i in range(H + 2):
                    if i < H:
                        pend.append((i, emit_scores(pi, i)))
                    if i >= 2:
                        hh_, eb_ = pend.popleft()
                        emit_tail(pi, hh_, eb_, catT)
                    if pending_out:
                        pending_out.popleft()()  # prev pair's out-proj
                    for _ in range(2):
                        if fillers:
                            fillers.popleft()()
                while fillers:
                    fillers.popleft()()
                pending_out.extend(mk_outproj(pi, catT))
                state.pop(pi - 1, None)
            while pending_out:
                pending_out.popleft()()

    nc.compile()
    return nc


_CACHE = {}
BENCH_LO, BENCH_HI = 3, 11


def _make_exec(iters):
    """Build the bass module for `iters` pipeline repetitions and return a
    jitted SPMD dispatcher plus metadata."""
    from jax.sharding import Mesh, PartitionSpec
    from jax.experimental.shard_map import shard_map
    from concourse.bass2jax import (
        _bass_exec_p, install_neuronx_cc_hook, partition_id_tensor)
    import concourse.mybir as mybir_

    nc = bacc.Bacc("TRN2", target_bir_lowering=False, debug=False)
    _emit(nc, iters=iters)

    install_neuronx_cc_hook()

    partition_name = (nc.partition_id_tensor.name
                      if nc.partition_id_tensor else None)
    in_names, out_names, out_avals, zero_outs = [], [], [], []
    for alloc in nc.m.functions[0].allocations:
        if not isinstance(alloc, mybir_.MemoryLocationSet):
            continue
        name = alloc.memorylocations[0].name
        if alloc.kind == "ExternalInput":
            if name != partition_name:
                in_names.append(name)
        elif alloc.kind == "ExternalOutput":
            out_names.append(name)
            shape = tuple(alloc.tensor_shape)
            dtype = mybir_.dt.np(alloc.dtype)
            out_avals.append(jax.core.ShapedArray(shape, dtype))
            zero_outs.append(np.zeros((NCORES * shape[0], *shape[1:]), dtype))
    n_params = len(in_names)
    all_names = in_names + out_names
    if partition_name is not None:
        all_names = all_names + [partition_name]

    def _body(*args):
        operands = list(args)
        if partition_name is not None:
            operands.append(partition_id_tensor())
        outs = _bass_exec_p.bind(
            *operands,
            out_avals=tuple(out_avals),
            in_names=tuple(all_names),
            out_names=tuple(out_names),
            lowering_input_output_aliases=(),
            sim_require_finite=True,
            sim_require_nnan=True,
            nc=nc,
        )
        return tuple(outs)

    devices = jax.devices()[:NCORES]
    mesh = Mesh(np.asarray(devices), ("core",))
    n_outs = len(out_names)
    # x is batch-sharded; weights are replicated (sent once, not 8x)
    spec_of = {n: (PartitionSpec("core") if n == "x" else PartitionSpec())
               for n in in_names}
    nodonate = jax.jit(
        shard_map(
            _body, mesh=mesh,
            in_specs=tuple(spec_of[n] for n in in_names)
            + (PartitionSpec("core"),) * n_outs,
            out_specs=(PartitionSpec("core"),) * n_outs,
            check_rep=False,
        ),
        keep_unused=True,
    )
    return {
        "nc": nc, "exec": nodonate, "in_names": in_names,
        "out_names": out_names, "zero_outs": zero_outs,
        "mesh": mesh, "spec_of": spec_of,
    }


def _get_exec(iters):
    key = ("exec", iters)
    if key not in _CACHE:
        _CACHE[key] = _make_exec(iters)
    return _CACHE[key]


def _device_args(ex, in_map_global):
    from jax.sharding import NamedSharding, PartitionSpec
    args = [jax.device_put(in_map_global[n],
                           NamedSharding(ex["mesh"], ex["spec_of"][n]))
            for n in ex["in_names"]]
    zs = [jax.device_put(z, NamedSharding(ex["mesh"], PartitionSpec("core")))
          for z in ex["zero_outs"]]
    return args, zs


def _time_exec(ex, args, zs, iters=20, reps=8):
    """Min pipelined per-dispatch wall time with device-resident buffers."""
    import time as _t
    f = ex["exec"]
    for _ in range(3):
        o = f(*args, *zs)
        jax.block_until_ready(o)
    runs = []
    for _ in range(reps):
        t0 = _t.perf_counter()
        for _ in range(iters):
            o = f(*args, *zs)
        jax.block_until_ready(o)
        runs.append((_t.perf_counter() - t0) / iters)
    return min(runs), runs


def bench_hw(in_map_global, sizes=(BENCH_LO, BENCH_HI), iters=20,
             rounds=16):
    """True per-iteration HW kernel time: time NEFFs that run the whole
    pipeline n times back-to-back on-device for several n and take the
    least-squares slope of wall time vs n.  The slope cancels the axon
    dispatch overhead (~4.4 ms per dispatch, kernel-independent) and
    once-per-NEFF costs (weight loads, final drain/barrier).  The NEFFs are
    timed in interleaved rounds so slow drift in the RPC floor cancels out
    of the slope; per-size minima over all rounds are used.  The 1x NEFF is
    deliberately excluded: its device time partially hides under the
    dispatch pipeline, which would bias the slope optimistically."""
    import time as _t
    exs = [(n, _get_exec(n)) for n in sizes]
    prepped = [(n, ex, *_device_args(ex, in_map_global)) for n, ex in exs]

    def once(ex, args, zs):
        f = ex["exec"]
        t0 = _t.perf_counter()
        for _ in range(iters):
            o = f(*args, *zs)
        jax.block_until_ready(o)
        return (_t.perf_counter() - t0) / iters

    for n, ex, args, zs in prepped:
        for _ in range(3):
            jax.block_until_ready(ex["exec"](*args, *zs))
    runs = {n: [] for n in sizes}
    slopes = []
    for _ in range(rounds):
        vals = {}
        for n, ex, args, zs in prepped:
            vals[n] = once(ex, args, zs)
            runs[n].append(vals[n])
        ns = sorted(sizes)
        slopes.append((vals[ns[-1]] - vals[ns[0]]) / (ns[-1] - ns[0]))
    # median of per-round slopes: each round times all NEFF sizes
    # back-to-back, so slow drift of the RPC floor cancels within the
    # round, and the median rejects outlier rounds
    per = float(np.median(slopes))
    mins = {n: min(rs) for n, rs in runs.items()}
    return per, {"mins": mins, "runs": runs, "sizes": sizes,
                 "slopes": slopes}


def _run(in_map_global):
    ex = _get_exec(1)
    args, zs = _device_args(ex, in_map_global)
    outs = ex["exec"](*args, *zs)
    return {n: np.asarray(outs[i]) for i, n in enumerate(ex["out_names"])}


def kernel(x, Wq, Wk, Wv, Wo, bo):
    in_map = {
        "x": np.ascontiguousarray(np.asarray(x, np.float32)),      # [128,256,512]
        "Wq": np.asarray(Wq, np.float32),
        "Wk": np.asarray(Wk, np.float32),
        "Wv": np.asarray(Wv, np.float32),
        "Wo": np.asarray(Wo, np.float32),
        "bo": np.asarray(bo, np.float32).reshape(1, D),
    }
    out = _run(in_map)["out"]                                      # [128,256,512]
    return out.astype(np.float32)
